# revision 42
# baseline (speedup 1.0000x reference)
"""Distributed GQA attention kernel for 8 TRN2 NeuronCores.

Problem: B=2, S=2048, D=1280, NH=16 q-heads, NKV=8 kv-heads, HD=80 (RoPE +
per-head QK RMSNorm, causal, GQA rep=2, o_proj).

Sharding: data-parallel over batch (cores 0-3 -> batch 0, cores 4-7 ->
batch 1) x tensor-parallel over kv-head groups (4 groups per batch; each
core owns 4 q heads + 2 kv heads).  Each core computes a partial o_proj
(row-shard of Wo); the host sums the 4 partials per batch (the "all-reduce")
and stacks the two batches.

Device-side design (per core, all matmul operands bf16, f32 accumulation):
  - host pre-transposes x -> xT so the contraction dim (D) is on partitions
  - dense natural-layout QKV projection: psum[s,480qk]/[s,160v]
  - RMS stats via ACT square + DVE grouped reduce; RoPE via free-axis
    tensor ops with host-precomputed tables (norm weights folded into the
    tables; the rms scale is applied after RoPE - it commutes).
  - q/k transposed per head via PE transpose -> qT/kT [80, S] layouts
  - scores^T [kv, q] = K @ Q^T per (head, q-tile 512, kv-tile 128); exp with
    fused 1/sqrt(HD) scale on ScalarE straight out of PSUM (no max
    subtraction needed: RMS-normed rows bound |scores/sqrt(HD)| <= sqrt(80))
  - causal handled by multiplying the diagonal-band region with a
    precomputed 0/1 mask after exp
  - PV with V augmented by a ones column -> softmax denominator rides along
  - normalize via DVE reciprocal + GPSIMD partition_broadcast
  - o_proj accumulates 4 heads into psum, DMA'd straight to DRAM
"""

import numpy as np
import ml_dtypes

B, S, D = 2, 2048, 1280
NH, NKV, HD = 16, 8, 80
REP = NH // NKV
EPS = 1e-6
THETA = 1e6
NCORES = 8
GROUPS = 4          # TP groups per batch
QH = NH // GROUPS   # 4 q heads per core
KH = NKV // GROUPS  # 2 kv heads per core
NT = S // 128       # 16 s-tiles
NJ = S // 512       # 4 q-tiles
SCALE = 1.0 / float(np.sqrt(HD))

BF16 = ml_dtypes.bfloat16
F8 = ml_dtypes.float8_e4m3


def _build_host_consts(Wq, Wk, Wv, Wo, q_norm_w, k_norm_w):
    """Per-TP-group weight shards + shared tables."""
    inv_freq = 1.0 / (THETA ** (np.arange(0, HD, 2, dtype=np.float64) / HD))
    t = np.arange(S, dtype=np.float64)
    freqs = np.outer(t, inv_freq)          # [S, 40]
    cos = np.cos(freqs).astype(np.float32)
    sin = np.sin(freqs).astype(np.float32)

    def rope_block(w):
        # [S, 160] = [C1|S1|C2|S2]; norm weight w folded in.
        c1 = cos * w[None, :40]
        s1 = sin * w[None, 40:]
        c2 = cos * w[None, 40:]
        s2 = sin * w[None, :40]
        return np.concatenate([c1, s1, c2, s2], axis=1)  # [S, 160]

    rq = np.tile(rope_block(q_norm_w), (1, QH)).astype(BF16)   # [S, 640]
    rk = np.tile(rope_block(k_norm_w), (1, KH)).astype(BF16)   # [S, 320]

    # band mask [128, 4, 512]: mb[p, r, c] = 1 if 128r + p <= c
    p = np.arange(128)[:, None, None]
    r = np.arange(4)[None, :, None]
    c = np.arange(512)[None, None, :]
    mb = ((128 * r + p) <= c).astype(BF16)                      # [128,4,512]

    ident = np.eye(128, dtype=BF16)

    shards = []
    for g in range(GROUPS):
        wq = Wq[:, g * QH * HD:(g + 1) * QH * HD]               # [D, 320]
        wk = Wk[:, g * KH * HD:(g + 1) * KH * HD]               # [D, 160]
        wv = Wv[:, g * KH * HD:(g + 1) * KH * HD]               # [D, 160]
        wqkv = np.concatenate([wq, wk, wv], axis=1).astype(BF16)  # [D, 640]
        wo_g = Wo[g * QH * HD:(g + 1) * QH * HD, :]             # [320, D]
        # -> [80, 4*1280]: head h block = Wo rows h*80..h*80+80
        wo = np.concatenate(
            [wo_g[h * HD:(h + 1) * HD, :] for h in range(QH)], axis=1
        ).astype(BF16)                                          # [80, 5120]
        shards.append((wqkv, wo))
    return rq, rk, mb, ident, shards


def _build_graph():
    import concourse.bacc as bacc
    import concourse.mybir as mybir
    from concourse.tile import TileContext

    f32 = mybir.dt.float32
    bf16 = mybir.dt.bfloat16
    f8 = mybir.dt.float8e4
    AF = mybir.ActivationFunctionType

    nc = bacc.Bacc("TRN2", target_bir_lowering=False, debug=False,
                   num_devices=NCORES)

    xT_d = nc.dram_tensor("xT", [10, 128, S], bf16, kind="ExternalInput")
    wqkv_d = nc.dram_tensor("wqkv", [10, 128, 640], bf16, kind="ExternalInput")
    wo_d = nc.dram_tensor("wo", [HD, QH * D], bf16, kind="ExternalInput")
    rq_d = nc.dram_tensor("ropeq", [128, NT * 640], bf16, kind="ExternalInput")
    rk_d = nc.dram_tensor("ropek", [128, NT * 320], bf16, kind="ExternalInput")
    mb_d = nc.dram_tensor("maskband", [128, 2048], bf16, kind="ExternalInput")
    id_d = nc.dram_tensor("ident", [128, 128], bf16, kind="ExternalInput")
    out_d = nc.dram_tensor("out", [S, D], f32, kind="ExternalOutput")

    with TileContext(nc) as tc:
        with (
            tc.tile_pool(name="const", bufs=1) as cp,
            tc.tile_pool(name="persist", bufs=1) as pp,
        ):
            xT_sb = cp.tile([128, 10, S], bf16)
            w_sb = cp.tile([128, 10, 640], bf16)
            wo_sb = cp.tile([HD, QH, D], bf16)
            rq_sb = cp.tile([128, NT, 640], bf16)
            rk_sb = cp.tile([128, NT, 320], bf16)
            mb_sb = cp.tile([128, 4, 512], bf16)
            id_sb = cp.tile([128, 128], bf16)
            eps_sb = cp.tile([128, 1], f32)
            ones80_sb = cp.tile([1, HD], bf16)

            qT_sb = pp.tile([128, QH, S], bf16)
            kT_sb = pp.tile([128, KH, S], bf16)
            v_sb = pp.tile([128, NT, KH, 97], bf16)

            # spread input loads across the three HWDGE issue queues and
            # stream xT block-major so early s-tiles unblock quickly
            for c in range(10):
                nc.sync.dma_start(out=w_sb[:, c, :], in_=wqkv_d[c])
                nc.sync.dma_start(out=xT_sb[:, c, :], in_=xT_d[c])
            nc.sync.dma_start(out=wo_sb[:], in_=wo_d[:])
            nc.sync.dma_start(out=rq_sb[:], in_=rq_d[:])
            nc.sync.dma_start(out=rk_sb[:], in_=rk_d[:])
            nc.sync.dma_start(out=mb_sb[:], in_=mb_d[:])
            nc.sync.dma_start(out=id_sb[:], in_=id_d[:])
            nc.vector.memset(eps_sb[:], EPS)
            nc.vector.memset(ones80_sb[:], 1.0)
            nc.vector.memset(v_sb[:, :, :, HD:97], 0.0)
            nc.vector.memset(v_sb[:, :, :, 96:97], 1.0)

            # ---------------- phase 1: QKV projection + norm + rope ----------
            with (
                tc.tile_pool(name="p1ps", bufs=2, space="PSUM") as ps1,
                tc.tile_pool(name="p1w", bufs=4) as wp,
            ):
                for t in range(NT):
                    qk_ps = ps1.tile([128, 480], f32, tag="qk")
                    v_ps = ps1.tile([128, 160], f32, tag="v")
                    for c in range(10):
                        lhs = xT_sb[:, c, 128 * t:128 * (t + 1)]
                        nc.tensor.matmul(qk_ps[:], lhs, w_sb[:, c, 0:480],
                                         start=(c == 0), stop=(c == 9))
                        nc.tensor.matmul(v_ps[:], lhs, w_sb[:, c, 480:640],
                                         start=(c == 0), stop=(c == 9))
                    # v -> augmented V (ones col pre-set)
                    nc.scalar.copy(
                        v_sb[:, t, :, 0:HD],
                        v_ps.rearrange("p (g d) -> p g d", g=KH),
                    )
                    # rms stats
                    qk_nat = wp.tile([128, 480], bf16, tag="qknat")
                    nc.scalar.copy(qk_nat[:], qk_ps[:])
                    sq = wp.tile([128, 480], f32, tag="sq")
                    nc.scalar.activation(sq[:], qk_ps[:], AF.Square)
                    ssum = wp.tile([128, 6], f32, tag="ssum")
                    nc.vector.tensor_reduce(
                        ssum[:], sq.rearrange("p (h d) -> p h d", d=HD),
                        axis=mybir.AxisListType.X, op=mybir.AluOpType.add)
                    rmsq = wp.tile([128, 6], f32, tag="rmsq")
                    nc.scalar.activation(rmsq[:], ssum[:], AF.Sqrt,
                                         scale=1.0 / HD, bias=eps_sb[:])
                    rms = wp.tile([128, 6], f32, tag="rms")
                    nc.vector.reciprocal(rms[:], rmsq[:])

                    # rope (tables carry the norm weights)
                    qk3 = qk_nat.rearrange("p (h d) -> p h d", d=HD)
                    rope = wp.tile([128, 480], bf16, tag="rope")
                    ro3 = rope.rearrange("p (h d) -> p h d", d=HD)
                    tq1 = wp.tile([128, QH, 40], bf16, tag="tq1")
                    tq2 = wp.tile([128, QH, 40], bf16, tag="tq2")
                    tk1 = wp.tile([128, KH, 40], bf16, tag="tk1")
                    tk2 = wp.tile([128, KH, 40], bf16, tag="tk2")
                    rqt = rq_sb[:, t, :].rearrange("p (h four d) -> p h four d",
                                                   four=4, d=40)
                    rkt = rk_sb[:, t, :].rearrange("p (h four d) -> p h four d",
                                                   four=4, d=40)
                    # q halves
                    nc.vector.tensor_mul(tq1[:], qk3[:, 0:QH, 0:40], rqt[:, :, 0, :])
                    nc.vector.tensor_mul(tq2[:], qk3[:, 0:QH, 40:HD], rqt[:, :, 1, :])
                    nc.vector.tensor_sub(ro3[:, 0:QH, 0:40], tq1[:], tq2[:])
                    nc.vector.tensor_mul(tq1[:], qk3[:, 0:QH, 40:HD], rqt[:, :, 2, :])
                    nc.vector.tensor_mul(tq2[:], qk3[:, 0:QH, 0:40], rqt[:, :, 3, :])
                    nc.vector.tensor_add(ro3[:, 0:QH, 40:HD], tq1[:], tq2[:])
                    # k halves on gpsimd (frees DVE, the phase-1 bottleneck)
                    kof = QH
                    nc.gpsimd.tensor_mul(tk1[:], qk3[:, kof:kof + KH, 0:40], rkt[:, :, 0, :])
                    nc.gpsimd.tensor_mul(tk2[:], qk3[:, kof:kof + KH, 40:HD], rkt[:, :, 1, :])
                    nc.gpsimd.tensor_sub(ro3[:, kof:kof + KH, 0:40], tk1[:], tk2[:])
                    nc.gpsimd.tensor_mul(tk1[:], qk3[:, kof:kof + KH, 40:HD], rkt[:, :, 2, :])
                    nc.gpsimd.tensor_mul(tk2[:], qk3[:, kof:kof + KH, 0:40], rkt[:, :, 3, :])
                    nc.gpsimd.tensor_add(ro3[:, kof:kof + KH, 40:HD], tk1[:], tk2[:])

                    # apply rms scale per head -> qn
                    qn = wp.tile([128, 6, 128], bf16, tag="qn")
                    qn3 = qn[:, :, 0:HD]
                    for h in range(6):
                        eng = nc.vector if h < QH else nc.gpsimd
                        eng.tensor_scalar_mul(qn3[:, h, :], ro3[:, h, :],
                                              rms[:, h:h + 1])
                    # transpose each head -> qT/kT (PE transpose, copies split
                    # between ACT and DVE)
                    for h in range(6):
                        tp = ps1.tile([HD, 128], bf16, tag="tp")
                        nc.tensor.transpose(tp[:], qn3[:, h, :], id_sb[:])
                        if h < QH:
                            dest = qT_sb[0:HD, h, 128 * t:128 * (t + 1)]
                        else:
                            dest = kT_sb[0:HD, h - QH, 128 * t:128 * (t + 1)]
                        if h % 2 == 0:
                            nc.vector.tensor_copy(dest, tp[:])
                        else:
                            nc.scalar.copy(dest, tp[:])

            # ---------------- phase 2: attention + o_proj --------------------
            with (
                tc.tile_pool(name="psc", bufs=2, space="PSUM") as psc,
                tc.tile_pool(name="ppv", bufs=2, space="PSUM") as ppv,
                tc.tile_pool(name="pop", bufs=2, space="PSUM") as pop,
                tc.tile_pool(name="slabp", bufs=3) as slabp,
                tc.tile_pool(name="attnp", bufs=8) as attnp,
                tc.tile_pool(name="smallp", bufs=3) as smallp,
            ):
                for j in range(NJ):
                    attns = []
                    for h in range(QH):
                        g2 = h // REP
                        ntiles = 4 * (j + 1)
                        slab = slabp.tile([128, NT, 512], bf16, tag="slab")
                        for i2 in range(0, ntiles, 2):
                            sc = psc.tile([128, 1024], f32, tag="sc")
                            for ii in range(2):
                                i = i2 + ii
                                r = i - 4 * j
                                c0 = 128 * r if r > 0 else 0
                                nc.tensor.matmul(
                                    sc[:, 512 * ii + c0:512 * (ii + 1)],
                                    kT_sb[0:HD, g2, 128 * i:128 * (i + 1)],
                                    qT_sb[0:HD, h, 512 * j + c0:512 * (j + 1)],
                                    start=True, stop=True)
                            rlo = i2 - 4 * j
                            e0 = 128 * rlo if rlo > 0 else 0
                            nc.scalar.activation(
                                slab[:, i2:i2 + 2, e0:512],
                                sc.rearrange("p (a b) -> p a b", a=2)[:, :, e0:512],
                                AF.Exp, scale=SCALE)
                        # causal diagonal-block masks (cols below the
                        # block are skipped by the narrowed PV reads)
                        for r in range(4):
                            i = 4 * j + r
                            nc.vector.tensor_mul(
                                slab[:, i, 128 * r:128 * (r + 1)],
                                slab[:, i, 128 * r:128 * (r + 1)],
                                mb_sb[:, r, 128 * r:128 * (r + 1)])
                        # PV with ones column -> denominator on row HD
                        pv = ppv.tile([97, 512], f32, tag="pv")
                        for i in range(ntiles):
                            r = i - 4 * j
                            c0 = 128 * r if r > 0 else 0
                            nc.tensor.matmul(pv[:, c0:512], v_sb[:, i, g2, :],
                                             slab[:, i, c0:512],
                                             start=(i == 0),
                                             stop=(i == ntiles - 1))
                        pvs = smallp.tile([97, 512], f32, tag="pvs")
                        nc.vector.tensor_copy(pvs[:], pv[:])
                        rd = smallp.tile([1, 512], bf16, tag="rd")
                        with nc.allow_low_precision("bf16 softmax denom reciprocal"):
                            nc.vector.reciprocal(rd[:], pvs[96:97, :])
                        # reuse the pv psum bank for the broadcast matmul
                        nc.tensor.matmul(pv[0:HD, :], ones80_sb[:], rd[:],
                                         start=True, stop=True)
                        attn = attnp.tile([HD, 512], bf16, tag="attn")
                        nc.vector.tensor_mul(attn[:], pvs[0:HD, :], pv[0:HD, :])
                        attns.append(attn)
                    # o_proj for this q-tile: accumulate the 4 heads
                    for st in range(4):
                        row0 = 512 * j + 128 * st
                        for nsl_i, (n0, nw) in enumerate(((0, 512), (512, 512), (1024, 256))):
                            if j == NJ - 1:
                                # last q-tile: nothing follows, so borrow the
                                # idle scores/pv banks for a deeper burst
                                k3 = (3 * st + nsl_i) % 3
                                if k3 == 0:
                                    op = pop.tile([128, nw], f32, tag="op")
                                elif k3 == 1:
                                    op = ppv.tile([128, nw], f32, tag="pv")
                                else:
                                    op = psc.tile([128, nw], f32, tag="sc")
                            else:
                                op = pop.tile([128, nw], f32, tag="op")
                            for h in range(QH):
                                nc.tensor.matmul(
                                    op[:],
                                    attns[h][:, 128 * st:128 * (st + 1)],
                                    wo_sb[:, h, n0:n0 + nw],
                                    start=(h == 0), stop=(h == QH - 1))
                            ob = smallp.tile([128, nw], f32, tag="ob", bufs=6)
                            nc.vector.tensor_copy(ob[:], op[:])
                            nc.sync.dma_start(
                                out=out_d[row0:row0 + 128, n0:n0 + nw],
                                in_=ob[:])
    return nc


_GRAPH_CACHE = {}


def _get_graph():
    if "nc" not in _GRAPH_CACHE:
        nc = _build_graph()
        nc.finalize()
        _GRAPH_CACHE["nc"] = nc
    return _GRAPH_CACHE["nc"]


def kernel(x, Wq, Wk, Wv, Wo, q_norm_w, k_norm_w, _trace=False):
    from concourse.bass_utils import run_bass_kernel_spmd

    x = np.asarray(x, dtype=np.float32)
    Wq = np.asarray(Wq, dtype=np.float32)
    Wk = np.asarray(Wk, dtype=np.float32)
    Wv = np.asarray(Wv, dtype=np.float32)
    Wo = np.asarray(Wo, dtype=np.float32)
    q_norm_w = np.asarray(q_norm_w, dtype=np.float32)
    k_norm_w = np.asarray(k_norm_w, dtype=np.float32)

    rq, rk, mb, ident, shards = _build_host_consts(Wq, Wk, Wv, Wo,
                                                   q_norm_w, k_norm_w)
    # partition-major: row p holds [t, f] blocks so the DMA is 2D contiguous
    rq = np.ascontiguousarray(
        rq.reshape(NT, 128, 640).transpose(1, 0, 2).reshape(128, NT * 640))
    rk = np.ascontiguousarray(
        rk.reshape(NT, 128, 320).transpose(1, 0, 2).reshape(128, NT * 320))

    in_maps = []
    for core in range(NCORES):
        b = core // GROUPS
        g = core % GROUPS
        wqkv, wo = shards[g]
        xT = np.ascontiguousarray(x[b].T.astype(BF16)).reshape(10, 128, S)
        in_maps.append({
            "xT": xT,
            "wqkv": np.ascontiguousarray(wqkv.reshape(10, 128, 640)),
            "wo": wo,
            "ropeq": rq,
            "ropek": rk,
            "maskband": np.ascontiguousarray(mb.reshape(128, 2048)),
            "ident": ident,
        })

    nc = _get_graph()
    res = run_bass_kernel_spmd(nc, in_maps, core_ids=list(range(NCORES)),
                               trace=_trace)
    outs = [r["out"] for r in res.results]
    full = np.zeros((B, S, D), dtype=np.float32)
    for core in range(NCORES):
        full[core // GROUPS] += outs[core]
    if _trace:
        kernel.last_results = res
    return full
